# revision 20
# baseline (speedup 1.0000x reference)
"""CISS-VAE (per-cluster MoE-routed MLP chain) Trainium2 kernel.

Strategy (routing done on host, compute on device):
  - Rows are grouped by cluster label on the host. Core c processes all rows
    of cluster c (C == n_cores == 8), so every GEMM on the device is a dense
    per-cluster GEMM — this removes the 8x redundant compute the reference
    does (einsum over all clusters then select).
  - All tensors on device are feature-major ([features, rows]): weights W are
    used directly as matmul lhsT ([f_in(K), f_out(M)]), activations are the
    moving operand ([K, rows_block]). x/eps are transposed on the host.
  - Matmul operands are bf16 (full PE rate + fast weight load), accumulation
    is fp32 in PSUM; per-feature biases live on partitions and are fused into
    the PSUM->SBUF eviction (Relu/Identity/Exp), split between the Scalar and
    Vector engines to balance load.
  - Row blocks are software-pipelined: the encoder of block b+1 is emitted
    before the decoder of block b so the PE never idles during the latent
    reparameterization (ACT/DVE) chain.
  - Weight DMAs are emitted just-in-time before their first use; x/weights go
    on the sync HWDGE queue, eps/bias on the scalar HWDGE queue, output
    stores on the gpsimd SWDGE queue.
"""

import ml_dtypes
import numpy as np

import concourse.bacc as bacc
import concourse.mybir as mybir
import concourse.tile as tile
from concourse import bass_utils

P = 128
D_IN, LAT, C = 512, 64, 8
H0, H1, H2 = 1024, 512, 256
N_CORES = 8
F32 = mybir.dt.float32
BF16 = mybir.dt.bfloat16
AF = mybir.ActivationFunctionType
ALU = mybir.AluOpType
BF16_NP = ml_dtypes.bfloat16


def _ceil_to(x, m):
    return ((x + m - 1) // m) * m


def _b2d(b):
    """[f] bias -> [128, n_mtiles] (partition-major per m-tile)."""
    f = b.shape[0]
    if f >= P:
        return np.ascontiguousarray(b.reshape(f // P, P).T.astype(np.float32))
    return np.ascontiguousarray(b.reshape(1, f).T.astype(np.float32))


# layer table: name -> (f_in, f_out)
LAYERS = dict(
    enc0=(D_IN, H0),
    encu=(H0, H1),
    enc2=(H1, H2),
    mu=(H2, LAT),
    lv=(H2, LAT),
    dec0=(LAT, H2),
    dec1=(H2, H1),
    dec2=(H1, H0),
    fin=(H0, D_IN),
)


def _build_module(npad, blocks):
    nc = bacc.Bacc("TRN2", target_bir_lowering=False, debug=False)

    dram = {}

    def din(name, shape, dt):
        dram[name] = nc.dram_tensor(name, list(shape), dt, kind="ExternalInput").ap()
        return dram[name]

    xT = din("xT", (D_IN, npad), BF16)
    epsT = din("epsT", (LAT, npad), F32)

    for name, (fi, fo) in LAYERS.items():
        din("w_" + name, (fi, fo), BF16)
        din("b_" + name, (P if fo >= P else fo, max(1, fo // P)), F32)

    outT = nc.dram_tensor("outT", [D_IN, npad], F32, kind="ExternalOutput").ap()

    with tile.TileContext(nc) as tc:
        with (
            tc.tile_pool(name="wpool", bufs=1) as wpool,
            tc.tile_pool(name="acts", bufs=2) as acts,
            tc.tile_pool(name="psum", bufs=7, space="PSUM") as psum,
        ):
            wsb = {}  # name -> list of [kp, f_out] tiles (loaded lazily)
            bsb = {}  # name -> [P or fo, n_m] tile
            dma_rr = [0]

            # The Scalar engine runs the critical-path PSUM evictions, and its
            # instruction queue is FIFO: a DMA issue that waits on a tile slot
            # would head-of-line-block evictions and stall the PE. So Scalar
            # only gets prologue DMAs (block-0 x / enc0 weights, which can
            # never wait), sync carries the steady-state loads, and gpsimd
            # (SWDGE) carries decoder weights and output stores.
            def prologue_dma(out, in_):
                eng = nc.sync if dma_rr[0] % 2 == 0 else nc.scalar
                dma_rr[0] += 1
                eng.dma_start(out, in_)

            DEC_W = ("dec0", "dec1", "dec2", "fin")

            def load_weights(name):
                if name in wsb:
                    return
                fi, fo = LAYERS[name]
                ktiles = []
                for k in range(max(1, fi // P)):
                    kp = min(P, fi)
                    w_t = wpool.tile([kp, fo], BF16, tag=f"w_{name}_{k}", name=f"w_{name}_{k}")
                    src_ap = dram["w_" + name][k * P : k * P + kp, :]
                    if name == "enc0":
                        prologue_dma(w_t[:], src_ap)
                    elif name in DEC_W:
                        nc.gpsimd.dma_start(w_t[:], src_ap)
                    else:
                        nc.sync.dma_start(w_t[:], src_ap)
                    ktiles.append(w_t)
                wsb[name] = ktiles
                bp = P if fo >= P else fo
                b_t = wpool.tile([bp, max(1, fo // P)], F32, tag=f"b_{name}", name=f"b_{name}")
                nc.gpsimd.dma_start(b_t[:], dram["b_" + name][:])
                bsb[name] = b_t

            def dense(lname, in_tiles, nb, func, scale=1.0, bufs=2, out_dt=BF16, evict="act", nsplit=1, ms=None):
                """out[f_out, nb] = func(scale*(W.T @ in) + b); returns m-tile list.

                evict="act": scalar-engine activation (any func).
                evict="dve": vector-engine tensor_scalar (Relu or Identity only).
                nsplit: split the rows dim into halves so thin layers pipeline
                their PSUM evictions with the following layer's matmuls.
                """
                load_weights(lname)
                fi, fo = LAYERS[lname]
                wk, bt = wsb[lname], bsb[lname]
                n_m = max(1, fo // P)
                n_k = len(wk)
                if nb % 512 or nsplit * 256 > nb:
                    nsplit = 1
                nh = nb // nsplit
                if ms is None:
                    ms = range(n_m)
                outs = []
                for m in ms:
                    mp = min(P, fo)
                    o_t = acts.tile(
                        [mp, nb], out_dt, tag=f"{lname}_{m}", bufs=bufs,
                        name=f"h_{lname}_{m}",
                    )
                    bias = bt[:mp, m : m + 1]
                    for h in range(nsplit):
                        sl = slice(h * nh, (h + 1) * nh)
                        ps = psum.tile([mp, nh], F32, tag="ps", name=f"ps_{lname}_{m}_{h}")
                        for k in range(n_k):
                            nc.tensor.matmul(
                                ps[:],
                                wk[k][:, m * mp : (m + 1) * mp],
                                in_tiles[k][:, sl],
                                start=(k == 0),
                                stop=(k == n_k - 1),
                            )
                        use_dve = evict == "dve" or (evict == "alt" and m % 2 == 1)
                        if use_dve:
                            if func is AF.Relu:
                                nc.vector.tensor_scalar(o_t[:, sl], ps[:], bias, 0.0, ALU.add, ALU.max)
                            else:  # Identity
                                nc.vector.tensor_scalar(o_t[:, sl], ps[:], bias, None, ALU.add)
                        else:
                            nc.scalar.activation(o_t[:, sl], ps[:], func, bias=bias, scale=scale)
                    outs.append(o_t)
                return outs

            n_blk = len(blocks)
            offs = [sum(blocks[:i]) for i in range(n_blk)]
            lat_out = [None] * n_blk  # z tiles per block
            x_in = [None] * n_blk
            eps_in = [None] * n_blk
            enc_out = [None] * n_blk

            def stage_load(b):
                nb, off = blocks[b], offs[b]
                x_tiles = []
                for k in range(D_IN // P):
                    x_t = acts.tile([P, nb], BF16, tag=f"x_{k}", bufs=3, name=f"x_{k}")
                    src_ap = xT[k * P : (k + 1) * P, off : off + nb]
                    if b == 0:
                        prologue_dma(x_t[:], src_ap)
                    else:
                        nc.sync.dma_start(x_t[:], src_ap)
                    x_tiles.append(x_t)
                e_t = acts.tile([LAT, nb], F32, tag="eps", bufs=3, name="e_t")
                (prologue_dma if b == 0 else nc.sync.dma_start)(e_t[:], epsT[:, off : off + nb])
                x_in[b], eps_in[b] = x_tiles, e_t

            h3_of = [None] * n_blk
            h4_of = [None] * n_blk

            def stage_encA(b):
                nb = blocks[b]
                h0 = dense("enc0", x_in[b], nb, AF.Relu, evict="alt")
                h1 = dense("encu", h0, nb, AF.Relu, evict="alt")
                enc_out[b] = dense("enc2", h1, nb, AF.Relu, evict="alt")

            def stage_mu(b):
                nb = blocks[b]
                h2 = enc_out[b]
                mu = dense("mu", h2, nb, AF.Identity, out_dt=F32, evict="dve")[0]
                sg = dense("lv", h2, nb, AF.Exp, scale=0.5, out_dt=F32)[0]
                enc_out[b] = (mu, sg)

            def stage_lat(b):
                nb = blocks[b]
                mu, sg = enc_out[b]
                tmp = acts.tile([LAT, nb], F32, tag="tmp", bufs=2, name="tmp")
                nc.vector.tensor_mul(tmp[:], sg[:], eps_in[b][:])
                z = acts.tile([LAT, nb], BF16, tag="z", bufs=2, name="z")
                nc.vector.tensor_add(z[:], tmp[:], mu[:])
                lat_out[b] = z

            h5_of = [None] * n_blk

            def stage_dec0(b):
                h3_of[b] = dense("dec0", [lat_out[b]], blocks[b], AF.Relu, evict="alt")

            def stage_dec1(b):
                h4_of[b] = dense("dec1", h3_of[b], blocks[b], AF.Relu, evict="alt")

            def stage_dec2(b):
                h5_of[b] = dense("dec2", h4_of[b], blocks[b], AF.Relu, evict="alt", bufs=3)

            def stage_fin(b, half):
                if b < 0:
                    return
                nb, off = blocks[b], offs[b]
                n_m = D_IN // P
                ms = range(n_m // 2) if half == 0 else range(n_m // 2, n_m)
                ot = dense("fin", h5_of[b], nb, AF.Identity, out_dt=F32, evict="alt", ms=ms)
                for i, m in enumerate(ms):
                    nc.sync.dma_start(outT[m * P : (m + 1) * P, off : off + nb], ot[i][:])

            # 3-deep software pipeline: decoder stages of block b-1 and the
            # final layer of block b-2 are interleaved between the thin
            # encoder stages of block b so the PE always has fat matmul work
            # while PSUM evictions / the latent chain complete.
            stage_load(0)
            stage_encA(0)
            stage_mu(0)
            stage_lat(0)
            for b in range(1, n_blk):
                stage_load(b)
                stage_encA(b)
                stage_dec0(b - 1)
                stage_fin(b - 2, 0)
                stage_mu(b)
                stage_lat(b)
                stage_dec1(b - 1)
                stage_fin(b - 2, 1)
                stage_dec2(b - 1)
            stage_dec0(n_blk - 1)
            stage_fin(n_blk - 2, 0)
            stage_dec1(n_blk - 1)
            stage_fin(n_blk - 2, 1)
            stage_dec2(n_blk - 1)
            stage_fin(n_blk - 1, 0)
            stage_fin(n_blk - 1, 1)

    nc.compile()
    return nc


def kernel(**inputs):
    x = np.asarray(inputs["x"], dtype=np.float32)
    lbl = np.asarray(inputs["cluster_labels"]).astype(np.int64)
    eps = np.asarray(inputs["eps"], dtype=np.float32)
    B = x.shape[0]

    counts = np.bincount(lbl, minlength=C)
    npad = max(512, _ceil_to(int(counts.max()), 64))
    n_full, rem = divmod(npad, 512)
    blocks = [512] * n_full + ([rem] if rem else [])

    rows = [np.nonzero(lbl == c)[0] for c in range(C)]

    def w16(a):
        return np.ascontiguousarray(np.asarray(a, dtype=np.float32).astype(BF16_NP))

    shared = {
        "w_enc0": w16(inputs["enc_W0"]),
        "b_enc0": _b2d(np.asarray(inputs["enc_b0"])),
        "w_enc2": w16(inputs["enc_W2"]),
        "b_enc2": _b2d(np.asarray(inputs["enc_b2"])),
        "w_mu": w16(inputs["mu_W"]),
        "b_mu": _b2d(np.asarray(inputs["mu_b"])),
        "w_lv": w16(inputs["lv_W"]),
        "b_lv": _b2d(0.5 * np.asarray(inputs["lv_b"])),
        "w_dec1": w16(inputs["dec_W1"]),
        "b_dec1": _b2d(np.asarray(inputs["dec_b1"])),
    }

    in_maps = []
    for c in range(C):
        r = rows[c]
        xT = np.zeros((D_IN, npad), BF16_NP)
        xT[:, : len(r)] = x[r].T.astype(BF16_NP)
        epsT = np.zeros((LAT, npad), np.float32)
        epsT[:, : len(r)] = eps[r].T
        m = dict(shared)
        m["xT"] = xT
        m["epsT"] = epsT
        m["w_encu"] = w16(inputs["enc_Wu"][c])
        m["b_encu"] = _b2d(np.asarray(inputs["enc_bu"][c]))
        m["w_dec0"] = w16(inputs["dec_Wu0"][c])
        m["b_dec0"] = _b2d(np.asarray(inputs["dec_bu0"][c]))
        m["w_dec2"] = w16(inputs["dec_Wu2"][c])
        m["b_dec2"] = _b2d(np.asarray(inputs["dec_bu2"][c]))
        m["w_fin"] = w16(inputs["fin_W"][c])
        m["b_fin"] = _b2d(np.asarray(inputs["fin_b"][c]))
        in_maps.append(m)

    nc = _build_module(npad, blocks)
    res = bass_utils.run_bass_kernel_spmd(nc, in_maps, core_ids=list(range(N_CORES)))
    global LAST_RESULTS
    LAST_RESULTS = res

    out = np.empty((B, D_IN), np.float32)
    for c in range(C):
        r = rows[c]
        out[r] = res.results[c]["outT"][:, : len(r)].T
    return out


# revision 21
# speedup vs baseline: 1.1803x; 1.1803x over previous
"""CISS-VAE (per-cluster MoE-routed MLP chain) Trainium2 kernel.

Strategy (routing done on host, compute on device):
  - Rows are grouped by cluster label on the host. Core c processes all rows
    of cluster c (C == n_cores == 8), so every GEMM on the device is a dense
    per-cluster GEMM — this removes the 8x redundant compute the reference
    does (einsum over all clusters then select).
  - All tensors on device are feature-major ([features, rows]): weights W are
    used directly as matmul lhsT ([f_in(K), f_out(M)]), activations are the
    moving operand ([K, rows_block]). x/eps are transposed on the host.
  - Matmul operands are bf16 (full PE rate + fast weight load), accumulation
    is fp32 in PSUM; per-feature biases live on partitions and are fused into
    the PSUM->SBUF eviction (Relu/Identity/Exp), split between the Scalar and
    Vector engines to balance load.
  - Row blocks are software-pipelined: the encoder of block b+1 is emitted
    before the decoder of block b so the PE never idles during the latent
    reparameterization (ACT/DVE) chain.
  - Weight DMAs are emitted just-in-time before their first use; x/weights go
    on the sync HWDGE queue, eps/bias on the scalar HWDGE queue, output
    stores on the gpsimd SWDGE queue.
"""

import ml_dtypes
import numpy as np

import concourse.bacc as bacc
import concourse.mybir as mybir
import concourse.tile as tile
from concourse import bass_utils

P = 128
D_IN, LAT, C = 512, 64, 8
H0, H1, H2 = 1024, 512, 256
N_CORES = 8
F32 = mybir.dt.float32
BF16 = mybir.dt.bfloat16
AF = mybir.ActivationFunctionType
ALU = mybir.AluOpType
BF16_NP = ml_dtypes.bfloat16


def _ceil_to(x, m):
    return ((x + m - 1) // m) * m


def _b2d(b):
    """[f] bias -> [128, n_mtiles] (partition-major per m-tile)."""
    f = b.shape[0]
    if f >= P:
        return np.ascontiguousarray(b.reshape(f // P, P).T.astype(np.float32))
    return np.ascontiguousarray(b.reshape(1, f).T.astype(np.float32))


# layer table: name -> (f_in, f_out)
LAYERS = dict(
    enc0=(D_IN, H0),
    encu=(H0, H1),
    enc2=(H1, H2),
    mu=(H2, LAT),
    lv=(H2, LAT),
    dec0=(LAT, H2),
    dec1=(H2, H1),
    dec2=(H1, H0),
    fin=(H0, D_IN),
)


def _build_module(npad, blocks):
    nc = bacc.Bacc("TRN2", target_bir_lowering=False, debug=False)

    dram = {}

    def din(name, shape, dt):
        dram[name] = nc.dram_tensor(name, list(shape), dt, kind="ExternalInput").ap()
        return dram[name]

    xT = din("xT", (D_IN, npad), BF16)
    epsT = din("epsT", (LAT, npad), F32)

    for name, (fi, fo) in LAYERS.items():
        din("w_" + name, (fi, fo), BF16)
        din("b_" + name, (P if fo >= P else fo, max(1, fo // P)), F32)

    outT = nc.dram_tensor("outT", [D_IN, npad], F32, kind="ExternalOutput").ap()

    with tile.TileContext(nc) as tc:
        with (
            tc.tile_pool(name="wpool", bufs=1) as wpool,
            tc.tile_pool(name="acts", bufs=2) as acts,
            tc.tile_pool(name="psum", bufs=7, space="PSUM") as psum,
        ):
            wsb = {}  # name -> list of [kp, f_out] tiles (loaded lazily)
            bsb = {}  # name -> [P or fo, n_m] tile
            dma_rr = [0]

            # The Scalar engine runs the critical-path PSUM evictions, and its
            # instruction queue is FIFO: a DMA issue that waits on a tile slot
            # would head-of-line-block evictions and stall the PE. So Scalar
            # only gets prologue DMAs (block-0 x / enc0 weights, which can
            # never wait), sync carries the steady-state loads, and gpsimd
            # (SWDGE) carries decoder weights and output stores.
            def prologue_dma(out, in_):
                eng = nc.sync if dma_rr[0] % 2 == 0 else nc.scalar
                dma_rr[0] += 1
                eng.dma_start(out, in_)

            DEC_W = ("dec0", "dec1", "dec2", "fin")

            def load_weights(name):
                if name in wsb:
                    return
                fi, fo = LAYERS[name]
                ktiles = []
                for k in range(max(1, fi // P)):
                    kp = min(P, fi)
                    w_t = wpool.tile([kp, fo], BF16, tag=f"w_{name}_{k}", name=f"w_{name}_{k}")
                    src_ap = dram["w_" + name][k * P : k * P + kp, :]
                    if name == "enc0":
                        prologue_dma(w_t[:], src_ap)
                    elif name in DEC_W:
                        nc.gpsimd.dma_start(w_t[:], src_ap)
                    else:
                        nc.sync.dma_start(w_t[:], src_ap)
                    ktiles.append(w_t)
                wsb[name] = ktiles
                bp = P if fo >= P else fo
                b_t = wpool.tile([bp, max(1, fo // P)], F32, tag=f"b_{name}", name=f"b_{name}")
                nc.gpsimd.dma_start(b_t[:], dram["b_" + name][:])
                bsb[name] = b_t

            def dense(lname, in_tiles, nb, func, scale=1.0, bufs=2, out_dt=BF16, evict="act", nsplit=1, ms=None):
                """out[f_out, nb] = func(scale*(W.T @ in) + b); returns m-tile list.

                evict="act": scalar-engine activation (any func).
                evict="dve": vector-engine tensor_scalar (Relu or Identity only).
                nsplit: split the rows dim into halves so thin layers pipeline
                their PSUM evictions with the following layer's matmuls.
                """
                load_weights(lname)
                fi, fo = LAYERS[lname]
                wk, bt = wsb[lname], bsb[lname]
                n_m = max(1, fo // P)
                n_k = len(wk)
                if nb % 512 or nsplit * 256 > nb:
                    nsplit = 1
                nh = nb // nsplit
                if ms is None:
                    ms = range(n_m)
                outs = []
                for m in ms:
                    mp = min(P, fo)
                    o_t = acts.tile(
                        [mp, nb], out_dt, tag=f"{lname}_{m}", bufs=bufs,
                        name=f"h_{lname}_{m}",
                    )
                    bias = bt[:mp, m : m + 1]
                    for h in range(nsplit):
                        sl = slice(h * nh, (h + 1) * nh)
                        ps = psum.tile([mp, nh], F32, tag="ps", name=f"ps_{lname}_{m}_{h}")
                        for k in range(n_k):
                            nc.tensor.matmul(
                                ps[:],
                                wk[k][:, m * mp : (m + 1) * mp],
                                in_tiles[k][:, sl],
                                start=(k == 0),
                                stop=(k == n_k - 1),
                            )
                        use_dve = evict == "dve" or (evict == "alt" and m % 2 == 1)
                        if use_dve:
                            if func is AF.Relu:
                                nc.vector.tensor_scalar(o_t[:, sl], ps[:], bias, 0.0, ALU.add, ALU.max)
                            else:  # Identity
                                nc.vector.tensor_scalar(o_t[:, sl], ps[:], bias, None, ALU.add)
                        else:
                            nc.scalar.activation(o_t[:, sl], ps[:], func, bias=bias, scale=scale)
                    outs.append(o_t)
                return outs

            n_blk = len(blocks)
            offs = [sum(blocks[:i]) for i in range(n_blk)]
            lat_out = [None] * n_blk  # z tiles per block
            x_in = [None] * n_blk
            eps_in = [None] * n_blk
            enc_out = [None] * n_blk

            def stage_load(b):
                nb, off = blocks[b], offs[b]
                x_tiles = []
                for k in range(D_IN // P):
                    x_t = acts.tile([P, nb], BF16, tag=f"x_{k}", bufs=3, name=f"x_{k}")
                    src_ap = xT[k * P : (k + 1) * P, off : off + nb]
                    if b == 0:
                        prologue_dma(x_t[:], src_ap)
                    else:
                        nc.sync.dma_start(x_t[:], src_ap)
                    x_tiles.append(x_t)
                e_t = acts.tile([LAT, nb], F32, tag="eps", bufs=3, name="e_t")
                (prologue_dma if b == 0 else nc.sync.dma_start)(e_t[:], epsT[:, off : off + nb])
                x_in[b], eps_in[b] = x_tiles, e_t

            h3_of = [None] * n_blk
            h4_of = [None] * n_blk

            def stage_encA(b):
                nb = blocks[b]
                h0 = dense("enc0", x_in[b], nb, AF.Relu, evict="alt")
                h1 = dense("encu", h0, nb, AF.Relu, evict="alt")
                enc_out[b] = dense("enc2", h1, nb, AF.Relu, evict="alt")

            def stage_mu(b):
                nb = blocks[b]
                h2 = enc_out[b]
                mu = dense("mu", h2, nb, AF.Identity, out_dt=F32, evict="dve")[0]
                sg = dense("lv", h2, nb, AF.Exp, scale=0.5, out_dt=F32)[0]
                enc_out[b] = (mu, sg)

            def stage_lat(b):
                nb = blocks[b]
                mu, sg = enc_out[b]
                tmp = acts.tile([LAT, nb], F32, tag="tmp", bufs=2, name="tmp")
                nc.vector.tensor_mul(tmp[:], sg[:], eps_in[b][:])
                z = acts.tile([LAT, nb], BF16, tag="z", bufs=2, name="z")
                nc.vector.tensor_add(z[:], tmp[:], mu[:])
                lat_out[b] = z

            def stage_dec0(b):
                h3_of[b] = dense("dec0", [lat_out[b]], blocks[b], AF.Relu, evict="alt")

            def stage_dec1(b):
                h4_of[b] = dense("dec1", h3_of[b], blocks[b], AF.Relu, evict="alt")

            def stage_dec2(b):
                nb, off = blocks[b], offs[b]
                h5 = dense("dec2", h4_of[b], nb, AF.Relu, evict="alt")
                ot = dense("fin", h5, nb, AF.Identity, out_dt=F32, evict="alt")
                for m in range(D_IN // P):
                    nc.sync.dma_start(outT[m * P : (m + 1) * P, off : off + nb], ot[m][:])

            # Warm up the PE (HAM clock gate) with dummy matmuls while the
            # prologue DMAs stream in: real matmuls then start at 2.4 GHz.
            wu_w = wpool.tile([P, P], BF16, tag="wu_w", name="wu_w")
            wu_x = wpool.tile([P, 512], BF16, tag="wu_x", name="wu_x")
            nc.vector.memset(wu_w[:], 0.0)
            nc.vector.memset(wu_x[:], 0.0)
            wu_ps = psum.tile([P, 512], F32, tag="wu_ps", bufs=1, name="wu_ps")
            for _ in range(20):
                nc.tensor.matmul(wu_ps[:], wu_w[:], wu_x[:], start=True, stop=True)

            # software pipeline: decoder stages of block b-1 are interleaved
            # between the thin encoder stages of block b so the PE always has
            # matmul work while PSUM evictions / the latent chain complete.
            stage_load(0)
            stage_encA(0)
            stage_mu(0)
            stage_lat(0)
            for b in range(1, n_blk):
                stage_load(b)
                stage_encA(b)
                stage_dec0(b - 1)
                stage_mu(b)
                stage_lat(b)
                stage_dec1(b - 1)
                stage_dec2(b - 1)
            stage_dec0(n_blk - 1)
            stage_dec1(n_blk - 1)
            stage_dec2(n_blk - 1)

    nc.compile()
    return nc


def kernel(**inputs):
    x = np.asarray(inputs["x"], dtype=np.float32)
    lbl = np.asarray(inputs["cluster_labels"]).astype(np.int64)
    eps = np.asarray(inputs["eps"], dtype=np.float32)
    B = x.shape[0]

    counts = np.bincount(lbl, minlength=C)
    npad = max(512, _ceil_to(int(counts.max()), 64))
    n_full, rem = divmod(npad, 512)
    blocks = [512] * n_full + ([rem] if rem else [])

    rows = [np.nonzero(lbl == c)[0] for c in range(C)]

    def w16(a):
        return np.ascontiguousarray(np.asarray(a, dtype=np.float32).astype(BF16_NP))

    shared = {
        "w_enc0": w16(inputs["enc_W0"]),
        "b_enc0": _b2d(np.asarray(inputs["enc_b0"])),
        "w_enc2": w16(inputs["enc_W2"]),
        "b_enc2": _b2d(np.asarray(inputs["enc_b2"])),
        "w_mu": w16(inputs["mu_W"]),
        "b_mu": _b2d(np.asarray(inputs["mu_b"])),
        "w_lv": w16(inputs["lv_W"]),
        "b_lv": _b2d(0.5 * np.asarray(inputs["lv_b"])),
        "w_dec1": w16(inputs["dec_W1"]),
        "b_dec1": _b2d(np.asarray(inputs["dec_b1"])),
    }

    in_maps = []
    for c in range(C):
        r = rows[c]
        xT = np.zeros((D_IN, npad), BF16_NP)
        xT[:, : len(r)] = x[r].T.astype(BF16_NP)
        epsT = np.zeros((LAT, npad), np.float32)
        epsT[:, : len(r)] = eps[r].T
        m = dict(shared)
        m["xT"] = xT
        m["epsT"] = epsT
        m["w_encu"] = w16(inputs["enc_Wu"][c])
        m["b_encu"] = _b2d(np.asarray(inputs["enc_bu"][c]))
        m["w_dec0"] = w16(inputs["dec_Wu0"][c])
        m["b_dec0"] = _b2d(np.asarray(inputs["dec_bu0"][c]))
        m["w_dec2"] = w16(inputs["dec_Wu2"][c])
        m["b_dec2"] = _b2d(np.asarray(inputs["dec_bu2"][c]))
        m["w_fin"] = w16(inputs["fin_W"][c])
        m["b_fin"] = _b2d(np.asarray(inputs["fin_b"][c]))
        in_maps.append(m)

    nc = _build_module(npad, blocks)
    res = bass_utils.run_bass_kernel_spmd(nc, in_maps, core_ids=list(range(N_CORES)))
    global LAST_RESULTS
    LAST_RESULTS = res

    out = np.empty((B, D_IN), np.float32)
    for c in range(C):
        r = rows[c]
        out[r] = res.results[c]["outT"][:, : len(r)].T
    return out
